# revision 13
# baseline (speedup 1.0000x reference)
"""Trainium2 Bass kernel for nn_PinnGenerator: 21 Jacobi iterations of a
variable-coefficient 5-point stencil, PE-assisted (float32r matmul shifts).

Reference math (per batch, inner grid 1022x1022, Dirichlet-0 boundary):
    w  = exp(mu * prev_pre);  y3 = 0.5*cross(w) + 2*w_in
    y' = (f_in*H^2 + 0.5*w_in*cross0(y) + 0.5*cross0(w_in*y)) / y3
which factors into the per-iteration form used here:
    y' = R * (hw2 * S(y) + S(v) + F2),   v = hw2 * y
  R = 2/y3, hw2 = w_in/4, F2 = f_in*H^2/2,  S = 4-neighbor sum w/ 0 boundary.

Layout per core: 5 overlapped PE tiles, tile j partition p = grid row
126j + p - 1 (partitions 0/127 are vertical halos refreshed per iteration by
2 SBUF->SBUF DMAs). Columns: inner col c at array col c (1024-wide padded).

Per iteration, per tile (two bank-aligned column halves per psum):
  PE (float32r, PSUM fp32): psum1 = S(y); psum2 = F2 + S(v)   [7 matmuls/half]
  DVE: q = hw2*psum1; q += psum2; y' = R*q (f32r write; 2 of 5 tiles on Pool)
  GPSIMD: v' = (R*hw2)*q  -- decoupled from y', so y'/v halos (4 small DMAs
  per tile pair, two HWDGE queues) and both products pipeline freely.
"""

import numpy as np

import concourse.bass as bass
import concourse.bacc as bacc
import concourse.tile as tile
from concourse import mybir
from concourse.bass_utils import run_bass_kernel_spmd

F32 = mybir.dt.float32
F32R = mybir.dt.float32r
AL = mybir.AluOpType
AF = mybir.ActivationFunctionType

G = 1024
NI = G - 2
H = 1.0 / (G - 1)
B = 4
NCORES = 8
P = 128
NT = 5              # PE tiles per core
TI = 126            # interior rows per tile
WIDE = 1024
OWNED = 511
HALF = 512          # cols per half-tile (bank-aligned)


def _build_program(niter: int) -> bass.Bass:
    nc = bacc.Bacc("TRN2", debug=False)

    ppc_d = nc.dram_tensor("ppc", [P, NT, WIDE], F32, kind="ExternalInput")
    f_d = nc.dram_tensor("fsl", [P, NT, WIDE], F32, kind="ExternalInput")
    y0_d = nc.dram_tensor("y0", [P, NT, WIDE], F32, kind="ExternalInput")
    ai_d = nc.dram_tensor("ai", [P, 2, P], F32, kind="ExternalInput")  # A, I
    mu_d = nc.dram_tensor("mub", [P, 1], F32, kind="ExternalInput")
    yout = nc.dram_tensor("yout", [P, NT, WIDE], F32, kind="ExternalOutput")

    with tile.TileContext(nc) as tc:
        with tc.tile_pool(name="main", bufs=1) as pool, \
             tc.tile_pool(name="qp", bufs=4) as qpool, \
             tc.tile_pool(name="ps1", bufs=2, space="PSUM") as ps1pool, \
             tc.tile_pool(name="ps2", bufs=2, space="PSUM") as ps2pool:
            ya = pool.tile([P, NT, WIDE], F32R, tag="ya")
            yb = pool.tile([P, NT, WIDE], F32R, tag="yb")
            v = pool.tile([P, NT, WIDE], F32R, tag="v")
            hw2 = pool.tile([P, NT, WIDE], F32, tag="hw2")
            R = pool.tile([P, NT, WIDE], F32, tag="R")
            F2 = pool.tile([P, NT, WIDE], F32R, tag="F2")
            wc = pool.tile([P, NT, WIDE], F32, tag="wc")
            y3 = pool.tile([P, NT, WIDE], F32, tag="y3")
            aif = pool.tile([P, 2, P], F32, tag="aif")
            ai = pool.tile([P, 2, P], F32R, tag="ai")
            bmu = pool.tile([P, 1], F32, tag="bmu")

            # ---- load inputs (all big DMAs up front, two HWDGE queues) ----
            nc.sync.dma_start(out=bmu[:], in_=mu_d[:])
            nc.sync.dma_start(out=aif[:], in_=ai_d[:])
            nc.vector.tensor_copy(ai[:], aif[:])   # round to f32r for PE
            A = ai[:, 0, :]
            I = ai[:, 1, :]

            nc.sync.dma_start(out=wc[:], in_=ppc_d[:])
            nc.scalar.dma_start(out=hw2[:], in_=f_d[:])   # f parked in hw2 buf
            nc.scalar.dma_start(out=R[:], in_=y0_d[:])    # y0 parked in R buf

            # y_a <- round(y0); y_b <- copy (initializes guards/pads)
            nc.vector.tensor_copy(ya[:], R[:])
            nc.vector.tensor_copy(yb[:], ya[:])

            # ---- w = exp(mu * pp) ----
            nc.scalar.activation(wc[:], wc[:], AF.Exp, bias=0.0,
                                 scale=bmu[:, 0:1])

            # f32r copy of w for the PE vertical sums (borrows the F2 buffer)
            wr = F2
            nc.vector.tensor_copy(wr[:], wc[:])

            # horizontal pair-sum at inner cols (w col c+1 <-> inner col c)
            nc.vector.tensor_add(
                y3[:, 0:5, 0:1022], wc[:, 0:5, 0:1022], wc[:, 0:5, 2:1024]
            )
            # vertical pair-sums via PE: psum = UD(w), add per tile
            for j in range(NT):
                pw = ps1pool.tile([P, 1024], F32, tag="p1")
                nc.tensor.matmul(pw[:, 0:512], A, wr[:, j, 1:513],
                                 start=True, stop=True)
                nc.tensor.matmul(pw[:, 512:1022], A, wr[:, j, 513:1023],
                                 start=True, stop=True)
                nc.vector.tensor_add(
                    y3[:, j, 0:1022], y3[:, j, 0:1022], pw[:, 0:1022]
                )
            # y3h = 0.25*cross + w_c  (= y3/2);  R = 1/y3h = 2/y3
            nc.vector.scalar_tensor_tensor(
                y3[:, 0:5, 0:1022], y3[:, 0:5, 0:1022], 0.25,
                wc[:, 0:5, 1:1023], op0=AL.mult, op1=AL.add,
            )
            # F2 = f * H^2/2 (overwrites wr after the UD matmuls; f32r write)
            nc.vector.tensor_scalar_mul(
                F2[:, 0:5, 1:1023], hw2[:, 0:5, 1:1023], 0.5 * H * H
            )
            # hw2 = w_c / 4 (array convention, aligned with wc)
            nc.vector.tensor_scalar_mul(
                hw2[:, 0:5, 1:1023], wc[:, 0:5, 1:1023], 0.25
            )
            # zero guard cols of hw2 (v' is computed full-width)
            nc.vector.memset(hw2[:, :, 0:1], 0.0)
            nc.vector.memset(hw2[:, :, 1023:1024], 0.0)
            # R = 1/y3h via 2-pass Newton approx (~2 ULP), scratch in wc
            nc.vector.reciprocal_approx_accurate(
                R[:, 0:5, 1:1023], y3[:, 0:5, 0:1022],
                scratch=wc[:, 0:5, 1:1023],
            )
            # Dirichlet: tile0 partition0 is grid row -1 and must stay zero;
            # zeroing its R makes every y' write there produce 0.
            nc.vector.memset(R[0:1, 0, :], 0.0)
            # RW = R*hw2 (for v' = RW*q); reuses the y3 buffer
            RW = y3
            nc.vector.tensor_mul(RW[:], R[:], hw2[:])
            # v0 = hw2*y0 (on Pool: frees DVE for the reciprocal)
            nc.gpsimd.tensor_mul(v[:], hw2[:], ya[:].bitcast(F32))

            ycur, ynxt = ya, yb
            for it in range(niter):
                for j in range(NT):
                    p1 = ps1pool.tile([P, 1024], F32, tag="p1")
                    p2 = ps2pool.tile([P, 1024], F32, tag="p2")
                    yj = ycur[:, j, :]
                    vj = v[:, j, :]
                    for h in range(2):
                        c0 = 0 if h == 0 else HALF      # psum col base (inner)
                        n = HALF if h == 0 else 1022 - HALF
                        # psum1 = S(y): UD + left + right (guard cols give 0 BC)
                        nc.tensor.matmul(p1[:, c0:c0 + n], A, yj[:, c0 + 1:c0 + n + 1],
                                         start=True, stop=False)
                        nc.tensor.matmul(p1[:, c0:c0 + n], I, yj[:, c0:c0 + n],
                                         start=False, stop=False)
                        nc.tensor.matmul(p1[:, c0:c0 + n], I, yj[:, c0 + 2:c0 + n + 2],
                                         start=False, stop=True)
                    for h in range(2):
                        c0 = 0 if h == 0 else HALF
                        n = HALF if h == 0 else 1022 - HALF
                        # psum2 = F2 + S(v)
                        nc.tensor.matmul(p2[:, c0:c0 + n], I, F2[:, j, c0 + 1:c0 + n + 1],
                                         start=True, stop=False)
                        nc.tensor.matmul(p2[:, c0:c0 + n], A, vj[:, c0 + 1:c0 + n + 1],
                                         start=False, stop=False)
                        nc.tensor.matmul(p2[:, c0:c0 + n], I, vj[:, c0:c0 + n],
                                         start=False, stop=False)
                        nc.tensor.matmul(p2[:, c0:c0 + n], I, vj[:, c0 + 2:c0 + n + 2],
                                         start=False, stop=True)
                    # DVE combine over the whole tile (psum spans 2 banks)
                    q = qpool.tile([P, 1024], F32, tag="q")
                    nc.vector.tensor_mul(
                        q[:, 0:1022], hw2[:, j, 1:1023], p1[:, 0:1022]
                    )
                    nc.vector.tensor_add(
                        q[:, 0:1022], q[:, 0:1022], p2[:, 0:1022]
                    )
                    if it != niter - 1:
                        # v' = RW*q directly (independent of y'; halo
                        # partitions fixed below by v-halo DMAs)
                        nc.gpsimd.tensor_mul(
                            v[:, j, 1:1023], RW[:, j, 1:1023], q[:, 0:1022]
                        )
                    if j in (1, 3) and it != niter - 1:
                        nc.gpsimd.tensor_mul(
                            ynxt[:, j, 1:1023], R[:, j, 1:1023], q[:, 0:1022]
                        )
                    else:
                        nc.vector.tensor_mul(
                            ynxt[:, j, 1:1023], R[:, j, 1:1023], q[:, 0:1022]
                        )
                if it != niter - 1:
                    # halo refresh for y' and v across tiles
                    for j in range(1, NT):
                        nc.sync.dma_start(out=ynxt[0:1, j, :],
                                          in_=ynxt[126:127, j - 1, :])
                        nc.scalar.dma_start(out=ynxt[127:128, j - 1, :],
                                            in_=ynxt[1:2, j, :])
                        nc.sync.dma_start(out=v[0:1, j, :],
                                          in_=v[126:127, j - 1, :])
                        nc.scalar.dma_start(out=v[127:128, j - 1, :],
                                            in_=v[1:2, j, :])
                ycur, ynxt = ynxt, ycur

            nc.sync.dma_start(out=yout[:], in_=ycur[:].bitcast(F32))

    nc.compile()
    return nc


_CACHE: dict = {}


def _g_of(L, half):
    return L if half == 0 else (NI - 1) - L


def _stage_core(pre_b, f_b, pp_b, half):
    p_idx = np.arange(P)[:, None]
    j_idx = np.arange(NT)[None, :]
    L = TI * j_idx + p_idx - 1                    # [P, NT]
    g = _g_of(L, half)
    fullrow = np.clip(g + 1, 0, G - 1)
    ppc = np.ascontiguousarray(pp_b[fullrow, :], dtype=np.float32)

    gc = np.clip(g, 0, NI - 1)
    f_sl = np.zeros((P, NT, WIDE), dtype=np.float32)
    f_sl[:, :, 1:1023] = f_b[gc + 1, 1:1023]

    y0 = np.zeros((P, NT, WIDE), dtype=np.float32)
    valid = (g >= 0) & (g < NI)
    vals = pre_b[gc, :]
    y0[:, :, 1:1023] = np.where(valid[:, :, None], vals, 0.0)
    return ppc, f_sl, y0


def kernel(pre, f, mu, prev_pre, maxiter, _want_results=False, _trace=False):
    pre = np.asarray(pre, dtype=np.float32)
    f = np.asarray(f, dtype=np.float32)
    prev_pre = np.asarray(prev_pre, dtype=np.float32)
    mu_val = float(np.asarray(mu).reshape(-1)[0])
    niter = int(maxiter) + 1

    if niter not in _CACHE:
        _CACHE[niter] = _build_program(niter)
    nc = _CACHE[niter]

    mu_arr = np.full((P, 1), mu_val, dtype=np.float32)
    ai = np.zeros((P, 2, P), dtype=np.float32)
    for i in range(P - 1):
        ai[i, 0, i + 1] = 1.0
        ai[i + 1, 0, i] = 1.0
    for i in range(P):
        ai[i, 1, i] = 1.0

    in_maps = []
    for core in range(NCORES):
        b, half = core // 2, core % 2
        ppc, f_sl, y0 = _stage_core(
            pre[b, 0], f[b, 0], prev_pre[b, 0], half
        )
        in_maps.append({
            "ppc": ppc, "fsl": f_sl, "y0": y0,
            "ai": ai, "mub": mu_arr,
        })

    res = run_bass_kernel_spmd(
        nc, in_maps, core_ids=list(range(NCORES)), trace=_trace
    )

    out = np.zeros((B, 1, NI, NI), dtype=np.float32)
    for core in range(NCORES):
        b, half = core // 2, core % 2
        arr = res.results[core]["yout"].reshape(P, NT, WIDE)
        # interior rows: partition 1..126 of tile j = grid row 126j+p-1
        rows = arr[1:127, :, 1:1023]              # [126, NT, 1022]
        rows = np.ascontiguousarray(rows.transpose(1, 0, 2)).reshape(TI * NT, NI)
        if half == 0:
            out[b, 0, 0:OWNED, :] = rows[0:OWNED]
        else:
            out[b, 0, OWNED:NI, :] = rows[0:OWNED][::-1]
    if _want_results:
        return out, res
    return out


# revision 14
# speedup vs baseline: 1.0180x; 1.0180x over previous
"""Trainium2 Bass kernel for nn_PinnGenerator: 21 Jacobi iterations of a
variable-coefficient 5-point stencil, PE-assisted (float32r matmul shifts).

Reference math (per batch, inner grid 1022x1022, Dirichlet-0 boundary):
    w  = exp(mu * prev_pre);  y3 = 0.5*cross(w) + 2*w_in
    y' = (f_in*H^2 + 0.5*w_in*cross0(y) + 0.5*cross0(w_in*y)) / y3
which factors into the per-iteration form used here:
    y' = R * (hw2 * S(y) + S(v) + F2),   v = hw2 * y
  R = 2/y3, hw2 = w_in/4, F2 = f_in*H^2/2,  S = 4-neighbor sum w/ 0 boundary.

Layout per core: 5 overlapped PE tiles, tile j partition p = grid row
126j + p - 1 (partitions 0/127 are vertical halos refreshed per iteration by
2 SBUF->SBUF DMAs). Columns: inner col c at array col c (1024-wide padded).

Per iteration, per tile (two bank-aligned column halves per psum):
  PE (float32r, PSUM fp32): psum1 = S(y); psum2 = F2 + S(v)   [7 matmuls/half]
  DVE: q = hw2*psum1; q += psum2; y' = R*q (f32r write; 2 of 5 tiles on Pool)
  GPSIMD: v' = (R*hw2)*q  -- decoupled from y', so y'/v halos (4 small DMAs
  per tile pair, two HWDGE queues) and both products pipeline freely.
"""

import numpy as np

import concourse.bass as bass
import concourse.bacc as bacc
import concourse.tile as tile
from concourse import mybir
from concourse.bass_utils import run_bass_kernel_spmd

F32 = mybir.dt.float32
F32R = mybir.dt.float32r
AL = mybir.AluOpType
AF = mybir.ActivationFunctionType

G = 1024
NI = G - 2
H = 1.0 / (G - 1)
B = 4
NCORES = 8
P = 128
NT = 5              # PE tiles per core
TI = 126            # interior rows per tile
WIDE = 1024
OWNED = 511
HALF = 512          # cols per half-tile (bank-aligned)


def _build_program(niter: int) -> bass.Bass:
    nc = bacc.Bacc("TRN2", debug=False)

    ppc_d = nc.dram_tensor("ppc", [P, NT, WIDE], F32, kind="ExternalInput")
    f_d = nc.dram_tensor("fsl", [P, NT, WIDE], F32, kind="ExternalInput")
    y0_d = nc.dram_tensor("y0", [P, NT, WIDE], F32, kind="ExternalInput")
    ai_d = nc.dram_tensor("ai", [P, 2, P], F32, kind="ExternalInput")  # A, I
    mu_d = nc.dram_tensor("mub", [P, 1], F32, kind="ExternalInput")
    yout = nc.dram_tensor("yout", [P, NT, WIDE], F32, kind="ExternalOutput")

    with tile.TileContext(nc) as tc:
        with tc.tile_pool(name="main", bufs=1) as pool, \
             tc.tile_pool(name="qp", bufs=4) as qpool, \
             tc.tile_pool(name="ps1", bufs=2, space="PSUM") as ps1pool, \
             tc.tile_pool(name="ps2", bufs=2, space="PSUM") as ps2pool:
            ya = pool.tile([P, NT, WIDE], F32R, tag="ya")
            yb = pool.tile([P, NT, WIDE], F32R, tag="yb")
            v = pool.tile([P, NT, WIDE], F32R, tag="v")
            hw2 = pool.tile([P, NT, WIDE], F32, tag="hw2")
            R = pool.tile([P, NT, WIDE], F32, tag="R")
            F2 = pool.tile([P, NT, WIDE], F32R, tag="F2")
            wc = pool.tile([P, NT, WIDE], F32, tag="wc")
            y3 = pool.tile([P, NT, WIDE], F32, tag="y3")
            aif = pool.tile([P, 2, P], F32, tag="aif")
            ai = pool.tile([P, 2, P], F32R, tag="ai")
            bmu = pool.tile([P, 1], F32, tag="bmu")

            # ---- load inputs (all big DMAs up front, two HWDGE queues) ----
            nc.sync.dma_start(out=bmu[:], in_=mu_d[:])
            nc.sync.dma_start(out=aif[:], in_=ai_d[:])
            nc.vector.tensor_copy(ai[:], aif[:])   # round to f32r for PE
            A = ai[:, 0, :]
            I = ai[:, 1, :]

            nc.sync.dma_start(out=wc[:], in_=ppc_d[:])
            nc.scalar.dma_start(out=hw2[:], in_=f_d[:])   # f parked in hw2 buf
            nc.scalar.dma_start(out=R[:], in_=y0_d[:])    # y0 parked in R buf

            # y_a <- round(y0); y_b <- copy (initializes guards/pads)
            nc.vector.tensor_copy(ya[:], R[:])
            nc.vector.tensor_copy(yb[:], ya[:])

            # ---- w = exp(mu * pp) ----
            nc.scalar.activation(wc[:], wc[:], AF.Exp, bias=0.0,
                                 scale=bmu[:, 0:1])

            # f32r copy of w for the PE vertical sums (borrows the F2 buffer)
            wr = F2
            nc.vector.tensor_copy(wr[:], wc[:])

            # horizontal pair-sum at inner cols (w col c+1 <-> inner col c)
            nc.vector.tensor_add(
                y3[:, 0:5, 0:1022], wc[:, 0:5, 0:1022], wc[:, 0:5, 2:1024]
            )
            # vertical pair-sums via PE: psum = UD(w), add per tile
            for j in range(NT):
                pw = ps1pool.tile([P, 1024], F32, tag="p1")
                nc.tensor.matmul(pw[:, 0:512], A, wr[:, j, 1:513],
                                 start=True, stop=True)
                nc.tensor.matmul(pw[:, 512:1022], A, wr[:, j, 513:1023],
                                 start=True, stop=True)
                nc.vector.tensor_add(
                    y3[:, j, 0:1022], y3[:, j, 0:1022], pw[:, 0:1022]
                )
            # y3h = 0.25*cross + w_c  (= y3/2);  R = 1/y3h = 2/y3
            nc.vector.scalar_tensor_tensor(
                y3[:, 0:5, 0:1022], y3[:, 0:5, 0:1022], 0.25,
                wc[:, 0:5, 1:1023], op0=AL.mult, op1=AL.add,
            )
            # F2 = f * H^2/2 (overwrites wr after the UD matmuls; f32r write)
            nc.vector.tensor_scalar_mul(
                F2[:, 0:5, 1:1023], hw2[:, 0:5, 1:1023], 0.5 * H * H
            )
            # hw2 = w_c / 4 (array convention, aligned with wc)
            nc.vector.tensor_scalar_mul(
                hw2[:, 0:5, 1:1023], wc[:, 0:5, 1:1023], 0.25
            )
            # zero guard cols of hw2 (v' is computed full-width)
            nc.vector.memset(hw2[:, :, 0:1], 0.0)
            nc.vector.memset(hw2[:, :, 1023:1024], 0.0)
            # R = 1/y3h via 1-pass Newton approx (~4e-6 rel; f32r noise is
            # ~100x larger so this is accuracy-neutral)
            nc.vector.reciprocal_approx_fast(
                R[:, 0:5, 1:1023], y3[:, 0:5, 0:1022]
            )
            # Dirichlet: tile0 partition0 is grid row -1 and must stay zero;
            # zeroing its R makes every y' write there produce 0.
            nc.vector.memset(R[0:1, 0, :], 0.0)
            # v0 = hw2*y0 (DVE: gates iteration-1 matmuls, keep it early)
            nc.vector.tensor_mul(v[:], hw2[:], ya[:].bitcast(F32))
            # RW = R*hw2 (for v' = RW*q; first consumer is ~1 iteration away,
            # so Pool computes it in parallel with iteration 1). y3 buf reused.
            RW = y3
            nc.gpsimd.tensor_mul(RW[:], R[:], hw2[:])

            ycur, ynxt = ya, yb
            for it in range(niter):
                for j in range(NT):
                    p1 = ps1pool.tile([P, 1024], F32, tag="p1")
                    p2 = ps2pool.tile([P, 1024], F32, tag="p2")
                    yj = ycur[:, j, :]
                    vj = v[:, j, :]
                    for h in range(2):
                        c0 = 0 if h == 0 else HALF      # psum col base (inner)
                        n = HALF if h == 0 else 1022 - HALF
                        # psum1 = S(y): UD + left + right (guard cols give 0 BC)
                        nc.tensor.matmul(p1[:, c0:c0 + n], A, yj[:, c0 + 1:c0 + n + 1],
                                         start=True, stop=False)
                        nc.tensor.matmul(p1[:, c0:c0 + n], I, yj[:, c0:c0 + n],
                                         start=False, stop=False)
                        nc.tensor.matmul(p1[:, c0:c0 + n], I, yj[:, c0 + 2:c0 + n + 2],
                                         start=False, stop=True)
                    for h in range(2):
                        c0 = 0 if h == 0 else HALF
                        n = HALF if h == 0 else 1022 - HALF
                        # psum2 = F2 + S(v)
                        nc.tensor.matmul(p2[:, c0:c0 + n], I, F2[:, j, c0 + 1:c0 + n + 1],
                                         start=True, stop=False)
                        nc.tensor.matmul(p2[:, c0:c0 + n], A, vj[:, c0 + 1:c0 + n + 1],
                                         start=False, stop=False)
                        nc.tensor.matmul(p2[:, c0:c0 + n], I, vj[:, c0:c0 + n],
                                         start=False, stop=False)
                        nc.tensor.matmul(p2[:, c0:c0 + n], I, vj[:, c0 + 2:c0 + n + 2],
                                         start=False, stop=True)
                    # DVE combine over the whole tile (psum spans 2 banks)
                    q = qpool.tile([P, 1024], F32, tag="q")
                    nc.vector.tensor_mul(
                        q[:, 0:1022], hw2[:, j, 1:1023], p1[:, 0:1022]
                    )
                    nc.vector.tensor_add(
                        q[:, 0:1022], q[:, 0:1022], p2[:, 0:1022]
                    )
                    if it != niter - 1:
                        # v' = RW*q directly (independent of y'; halo
                        # partitions fixed below by v-halo DMAs)
                        nc.gpsimd.tensor_mul(
                            v[:, j, 1:1023], RW[:, j, 1:1023], q[:, 0:1022]
                        )
                    if j in (1, 3):
                        nc.gpsimd.tensor_mul(
                            ynxt[:, j, 1:1023], R[:, j, 1:1023], q[:, 0:1022]
                        )
                    else:
                        nc.vector.tensor_mul(
                            ynxt[:, j, 1:1023], R[:, j, 1:1023], q[:, 0:1022]
                        )
                if it != niter - 1:
                    # halo refresh for y' and v across tiles
                    for j in range(1, NT):
                        nc.sync.dma_start(out=ynxt[0:1, j, :],
                                          in_=ynxt[126:127, j - 1, :])
                        nc.scalar.dma_start(out=ynxt[127:128, j - 1, :],
                                            in_=ynxt[1:2, j, :])
                        nc.sync.dma_start(out=v[0:1, j, :],
                                          in_=v[126:127, j - 1, :])
                        nc.scalar.dma_start(out=v[127:128, j - 1, :],
                                            in_=v[1:2, j, :])
                ycur, ynxt = ynxt, ycur

            for j in range(NT):
                eng = nc.sync if j % 2 == 0 else nc.scalar
                eng.dma_start(out=yout[:, j, :], in_=ycur[:, j, :].bitcast(F32))

    nc.compile()
    return nc


_CACHE: dict = {}


def _g_of(L, half):
    return L if half == 0 else (NI - 1) - L


def _stage_core(pre_b, f_b, pp_b, half):
    p_idx = np.arange(P)[:, None]
    j_idx = np.arange(NT)[None, :]
    L = TI * j_idx + p_idx - 1                    # [P, NT]
    g = _g_of(L, half)
    fullrow = np.clip(g + 1, 0, G - 1)
    ppc = np.ascontiguousarray(pp_b[fullrow, :], dtype=np.float32)

    gc = np.clip(g, 0, NI - 1)
    f_sl = np.zeros((P, NT, WIDE), dtype=np.float32)
    f_sl[:, :, 1:1023] = f_b[gc + 1, 1:1023]

    y0 = np.zeros((P, NT, WIDE), dtype=np.float32)
    valid = (g >= 0) & (g < NI)
    vals = pre_b[gc, :]
    y0[:, :, 1:1023] = np.where(valid[:, :, None], vals, 0.0)
    return ppc, f_sl, y0


def kernel(pre, f, mu, prev_pre, maxiter, _want_results=False, _trace=False):
    pre = np.asarray(pre, dtype=np.float32)
    f = np.asarray(f, dtype=np.float32)
    prev_pre = np.asarray(prev_pre, dtype=np.float32)
    mu_val = float(np.asarray(mu).reshape(-1)[0])
    niter = int(maxiter) + 1

    if niter not in _CACHE:
        _CACHE[niter] = _build_program(niter)
    nc = _CACHE[niter]

    mu_arr = np.full((P, 1), mu_val, dtype=np.float32)
    ai = np.zeros((P, 2, P), dtype=np.float32)
    for i in range(P - 1):
        ai[i, 0, i + 1] = 1.0
        ai[i + 1, 0, i] = 1.0
    for i in range(P):
        ai[i, 1, i] = 1.0

    in_maps = []
    for core in range(NCORES):
        b, half = core // 2, core % 2
        ppc, f_sl, y0 = _stage_core(
            pre[b, 0], f[b, 0], prev_pre[b, 0], half
        )
        in_maps.append({
            "ppc": ppc, "fsl": f_sl, "y0": y0,
            "ai": ai, "mub": mu_arr,
        })

    res = run_bass_kernel_spmd(
        nc, in_maps, core_ids=list(range(NCORES)), trace=_trace
    )

    out = np.zeros((B, 1, NI, NI), dtype=np.float32)
    for core in range(NCORES):
        b, half = core // 2, core % 2
        arr = res.results[core]["yout"].reshape(P, NT, WIDE)
        # interior rows: partition 1..126 of tile j = grid row 126j+p-1
        rows = arr[1:127, :, 1:1023]              # [126, NT, 1022]
        rows = np.ascontiguousarray(rows.transpose(1, 0, 2)).reshape(TI * NT, NI)
        if half == 0:
            out[b, 0, 0:OWNED, :] = rows[0:OWNED]
        else:
            out[b, 0, OWNED:NI, :] = rows[0:OWNED][::-1]
    if _want_results:
        return out, res
    return out


# revision 21
# speedup vs baseline: 1.0325x; 1.0143x over previous
"""Trainium2 Bass kernel for nn_PinnGenerator: 21 Jacobi iterations of a
variable-coefficient 5-point stencil, PE-assisted (float32r matmul shifts).

Reference math (per batch, inner grid 1022x1022, Dirichlet-0 boundary):
    w  = exp(mu * prev_pre);  y3 = 0.5*cross(w) + 2*w_in
    y' = (f_in*H^2 + 0.5*w_in*cross0(y) + 0.5*cross0(w_in*y)) / y3
which factors into the per-iteration form used here:
    y' = R * (hw2 * S(y) + S(v) + F2),   v = hw2 * y
  R = 2/y3, hw2 = w_in/4, F2 = f_in*H^2/2,  S = 4-neighbor sum w/ 0 boundary.

Layout per core: 5 overlapped PE tiles, tile j partition p = grid row
126j + p - 1 (partitions 0/127 are vertical halos refreshed per iteration by
2 SBUF->SBUF DMAs). Columns: inner col c at array col c (1024-wide padded).

Per iteration, per tile (two bank-aligned column halves per psum):
  PE (float32r, PSUM fp32): psum1 = S(y); psum2 = F2 + S(v)   [7 matmuls/half]
  DVE: q = hw2*psum1; q += psum2; y' = R*q (f32r write; 2 of 5 tiles on Pool)
  GPSIMD: v' = (R*hw2)*q  -- decoupled from y', so y'/v halos (4 small DMAs
  per tile pair, two HWDGE queues) and both products pipeline freely.
"""

import numpy as np

import concourse.bass as bass
import concourse.bacc as bacc
import concourse.tile as tile
from concourse import mybir
from concourse.bass_utils import run_bass_kernel_spmd

F32 = mybir.dt.float32
F32R = mybir.dt.float32r
AL = mybir.AluOpType
AF = mybir.ActivationFunctionType

G = 1024
NI = G - 2
H = 1.0 / (G - 1)
B = 4
NCORES = 8
P = 128
NT = 5              # PE tiles per core
TI = 126            # interior rows per tile
WIDE = 1024
OWNED = 511
HALF = 512          # cols per half-tile (bank-aligned)


def _build_program(niter: int) -> bass.Bass:
    nc = bacc.Bacc("TRN2", debug=False)

    ppc_d = nc.dram_tensor("ppc", [P, NT, WIDE], F32, kind="ExternalInput")
    f_d = nc.dram_tensor("fsl", [P, NT, WIDE], F32, kind="ExternalInput")
    y0_d = nc.dram_tensor("y0", [P, NT, WIDE], F32, kind="ExternalInput")
    ai_d = nc.dram_tensor("ai", [P, 2, P], F32, kind="ExternalInput")  # A, I
    mu_d = nc.dram_tensor("mub", [P, 1], F32, kind="ExternalInput")
    yout = nc.dram_tensor("yout", [P, NT, WIDE], F32, kind="ExternalOutput")

    with tile.TileContext(nc) as tc:
        with tc.tile_pool(name="main", bufs=1) as pool, \
             tc.tile_pool(name="qp", bufs=4) as qpool, \
             tc.tile_pool(name="ps1", bufs=2, space="PSUM") as ps1pool, \
             tc.tile_pool(name="ps2", bufs=2, space="PSUM") as ps2pool:
            ya = pool.tile([P, NT, WIDE], F32R, tag="ya")
            yb = pool.tile([P, NT, WIDE], F32R, tag="yb")
            v = pool.tile([P, NT, WIDE], F32R, tag="v")
            hw2 = pool.tile([P, NT, WIDE], F32, tag="hw2")
            R = pool.tile([P, NT, WIDE], F32, tag="R")
            F2 = pool.tile([P, NT, WIDE], F32R, tag="F2")
            wc = pool.tile([P, NT, WIDE], F32, tag="wc")
            y3 = pool.tile([P, NT, WIDE], F32, tag="y3")
            aif = pool.tile([P, 2, P], F32, tag="aif")
            ai = pool.tile([P, 2, P], F32R, tag="ai")
            bmu = pool.tile([P, 1], F32, tag="bmu")

            # ---- load inputs (all big DMAs up front, two HWDGE queues) ----
            nc.sync.dma_start(out=bmu[:], in_=mu_d[:])
            nc.sync.dma_start(out=aif[:], in_=ai_d[:])
            nc.vector.tensor_copy(ai[:], aif[:])   # round to f32r for PE
            A = ai[:, 0, :]
            I = ai[:, 1, :]

            nc.sync.dma_start(out=wc[:], in_=ppc_d[:])
            nc.scalar.dma_start(out=hw2[:], in_=f_d[:])   # f parked in hw2 buf
            nc.scalar.dma_start(out=R[:], in_=y0_d[:])    # y0 parked in R buf

            # y_a <- round(y0); y_b <- copy (initializes guards/pads)
            nc.vector.tensor_copy(ya[:], R[:])
            nc.vector.tensor_copy(yb[:], ya[:])

            # ---- w = exp(mu * pp) ----
            nc.scalar.activation(wc[:], wc[:], AF.Exp, bias=0.0,
                                 scale=bmu[:, 0:1])

            # f32r copy of w for the PE vertical sums (borrows the F2 buffer)
            wr = F2
            nc.vector.tensor_copy(wr[:], wc[:])

            # full cross-sum of w via PE (UD + LR, f32r), then per tile
            # y3h = 0.25*cross + w_c (= y3/2) in one fused STT
            for j in range(NT):
                pw = ps1pool.tile([P, 1024], F32, tag="p1")
                nc.tensor.matmul(pw[:, 0:512], A, wr[:, j, 1:513],
                                 start=True, stop=False)
                nc.tensor.matmul(pw[:, 0:512], I, wr[:, j, 0:512],
                                 start=False, stop=False)
                nc.tensor.matmul(pw[:, 0:512], I, wr[:, j, 2:514],
                                 start=False, stop=True)
                nc.tensor.matmul(pw[:, 512:1022], A, wr[:, j, 513:1023],
                                 start=True, stop=False)
                nc.tensor.matmul(pw[:, 512:1022], I, wr[:, j, 512:1022],
                                 start=False, stop=False)
                nc.tensor.matmul(pw[:, 512:1022], I, wr[:, j, 514:1024],
                                 start=False, stop=True)
                nc.vector.scalar_tensor_tensor(
                    y3[:, j, 0:1022], pw[:, 0:1022], 0.25,
                    wc[:, j, 1:1023], op0=AL.mult, op1=AL.add,
                )
            # F2 = f * H^2/2 (overwrites wr after the UD matmuls; f32r write)
            nc.vector.tensor_scalar_mul(
                F2[:, 0:5, 1:1023], hw2[:, 0:5, 1:1023], 0.5 * H * H
            )
            # hw2 = w_c / 4 (array convention, aligned with wc)
            nc.vector.tensor_scalar_mul(
                hw2[:, 0:5, 1:1023], wc[:, 0:5, 1:1023], 0.25
            )
            # zero guard cols of hw2 (v' is computed full-width)
            nc.vector.memset(hw2[:, :, 0:1], 0.0)
            nc.vector.memset(hw2[:, :, 1023:1024], 0.0)
            # R = 1/y3h via 1-pass Newton approx (~4e-6 rel; f32r noise is
            # ~100x larger so this is accuracy-neutral)
            nc.vector.reciprocal_approx_fast(
                R[:, 0:5, 1:1023], y3[:, 0:5, 0:1022]
            )
            # Dirichlet: tile0 partition0 is grid row -1 and must stay zero;
            # zeroing its R makes every y' write there produce 0.
            nc.vector.memset(R[0:1, 0, :], 0.0)
            # v0 = hw2*y0 (DVE: gates iteration-1 matmuls, keep it early)
            nc.vector.tensor_mul(v[:], hw2[:], ya[:].bitcast(F32))
            # RW = R*hw2 (for v' = RW*q; first consumer is ~1 iteration away,
            # so Pool computes it in parallel with iteration 1). y3 buf reused.
            RW = y3
            nc.gpsimd.tensor_mul(RW[:], R[:], hw2[:])

            ycur, ynxt = ya, yb
            for it in range(niter):
                for j in range(NT):
                    p1 = ps1pool.tile([P, 1024], F32, tag="p1")
                    p2 = ps2pool.tile([P, 1024], F32, tag="p2")
                    yj = ycur[:, j, :]
                    vj = v[:, j, :]
                    for h in range(2):
                        c0 = 0 if h == 0 else HALF      # psum col base (inner)
                        n = HALF if h == 0 else 1022 - HALF
                        # psum1 = S(y): UD + left + right (guard cols give 0 BC)
                        nc.tensor.matmul(p1[:, c0:c0 + n], A, yj[:, c0 + 1:c0 + n + 1],
                                         start=True, stop=False)
                        nc.tensor.matmul(p1[:, c0:c0 + n], I, yj[:, c0:c0 + n],
                                         start=False, stop=False)
                        nc.tensor.matmul(p1[:, c0:c0 + n], I, yj[:, c0 + 2:c0 + n + 2],
                                         start=False, stop=True)
                    for h in range(2):
                        c0 = 0 if h == 0 else HALF
                        n = HALF if h == 0 else 1022 - HALF
                        # psum2 = F2 + S(v)
                        nc.tensor.matmul(p2[:, c0:c0 + n], I, F2[:, j, c0 + 1:c0 + n + 1],
                                         start=True, stop=False)
                        nc.tensor.matmul(p2[:, c0:c0 + n], A, vj[:, c0 + 1:c0 + n + 1],
                                         start=False, stop=False)
                        nc.tensor.matmul(p2[:, c0:c0 + n], I, vj[:, c0:c0 + n],
                                         start=False, stop=False)
                        nc.tensor.matmul(p2[:, c0:c0 + n], I, vj[:, c0 + 2:c0 + n + 2],
                                         start=False, stop=True)
                    # DVE combine over the whole tile (psum spans 2 banks)
                    q = qpool.tile([P, 1024], F32, tag="q")
                    nc.vector.tensor_mul(
                        q[:, 0:1022], hw2[:, j, 1:1023], p1[:, 0:1022]
                    )
                    nc.vector.tensor_add(
                        q[:, 0:1022], q[:, 0:1022], p2[:, 0:1022]
                    )
                    if it != niter - 1:
                        # v' = RW*q directly (independent of y'; halo
                        # partitions fixed below by v-halo DMAs)
                        nc.gpsimd.tensor_mul(
                            v[:, j, 1:1023], RW[:, j, 1:1023], q[:, 0:1022]
                        )
                    if j in (1, 3):
                        nc.gpsimd.tensor_mul(
                            ynxt[:, j, 1:1023], R[:, j, 1:1023], q[:, 0:1022]
                        )
                    else:
                        nc.vector.tensor_mul(
                            ynxt[:, j, 1:1023], R[:, j, 1:1023], q[:, 0:1022]
                        )
                if it != niter - 1:
                    # halo refresh for y' and v across tiles
                    for j in range(1, NT):
                        nc.sync.dma_start(out=ynxt[0:1, j, :],
                                          in_=ynxt[126:127, j - 1, :])
                        nc.scalar.dma_start(out=ynxt[127:128, j - 1, :],
                                            in_=ynxt[1:2, j, :])
                        nc.sync.dma_start(out=v[0:1, j, :],
                                          in_=v[126:127, j - 1, :])
                        nc.scalar.dma_start(out=v[127:128, j - 1, :],
                                            in_=v[1:2, j, :])
                ycur, ynxt = ynxt, ycur

            for j in range(NT):
                eng = nc.sync if j % 2 == 0 else nc.scalar
                eng.dma_start(out=yout[:, j, :], in_=ycur[:, j, :].bitcast(F32))

    nc.compile()
    return nc


_CACHE: dict = {}


def _g_of(L, half):
    return L if half == 0 else (NI - 1) - L


def _stage_core(pre_b, f_b, pp_b, half):
    p_idx = np.arange(P)[:, None]
    j_idx = np.arange(NT)[None, :]
    L = TI * j_idx + p_idx - 1                    # [P, NT]
    g = _g_of(L, half)
    fullrow = np.clip(g + 1, 0, G - 1)
    ppc = np.ascontiguousarray(pp_b[fullrow, :], dtype=np.float32)

    gc = np.clip(g, 0, NI - 1)
    f_sl = np.zeros((P, NT, WIDE), dtype=np.float32)
    f_sl[:, :, 1:1023] = f_b[gc + 1, 1:1023]

    y0 = np.zeros((P, NT, WIDE), dtype=np.float32)
    valid = (g >= 0) & (g < NI)
    vals = pre_b[gc, :]
    y0[:, :, 1:1023] = np.where(valid[:, :, None], vals, 0.0)
    return ppc, f_sl, y0


def kernel(pre, f, mu, prev_pre, maxiter, _want_results=False, _trace=False):
    pre = np.asarray(pre, dtype=np.float32)
    f = np.asarray(f, dtype=np.float32)
    prev_pre = np.asarray(prev_pre, dtype=np.float32)
    mu_val = float(np.asarray(mu).reshape(-1)[0])
    niter = int(maxiter) + 1

    if niter not in _CACHE:
        _CACHE[niter] = _build_program(niter)
    nc = _CACHE[niter]

    mu_arr = np.full((P, 1), mu_val, dtype=np.float32)
    ai = np.zeros((P, 2, P), dtype=np.float32)
    for i in range(P - 1):
        ai[i, 0, i + 1] = 1.0
        ai[i + 1, 0, i] = 1.0
    for i in range(P):
        ai[i, 1, i] = 1.0

    in_maps = []
    for core in range(NCORES):
        b, half = core // 2, core % 2
        ppc, f_sl, y0 = _stage_core(
            pre[b, 0], f[b, 0], prev_pre[b, 0], half
        )
        in_maps.append({
            "ppc": ppc, "fsl": f_sl, "y0": y0,
            "ai": ai, "mub": mu_arr,
        })

    res = run_bass_kernel_spmd(
        nc, in_maps, core_ids=list(range(NCORES)), trace=_trace
    )

    out = np.zeros((B, 1, NI, NI), dtype=np.float32)
    for core in range(NCORES):
        b, half = core // 2, core % 2
        arr = res.results[core]["yout"].reshape(P, NT, WIDE)
        # interior rows: partition 1..126 of tile j = grid row 126j+p-1
        rows = arr[1:127, :, 1:1023]              # [126, NT, 1022]
        rows = np.ascontiguousarray(rows.transpose(1, 0, 2)).reshape(TI * NT, NI)
        if half == 0:
            out[b, 0, 0:OWNED, :] = rows[0:OWNED]
        else:
            out[b, 0, OWNED:NI, :] = rows[0:OWNED][::-1]
    if _want_results:
        return out, res
    return out
